# revision 36
# baseline (speedup 1.0000x reference)
"""Trainium2 Bass kernel for nn_AutocorrelationCorrelogram.

For nervegram [B=4, F=50, T=20000, C=2]: 300 periodic-Hann-windowed frames
of length 512 per (b,f,c) signal, circular autocorrelation via
Wiener-Khinchin (rfft -> |.|^2 -> irfft), relu, normalize by sqrt(zero
lag), keep 256 lags, mean over channels -> [4, 50, 300, 256].

Sharding: pure data parallel over the 200 (b,f) pairs -> 25 per core x 8
cores (SPMD, no collectives).

v3 design (bf16, host-side frame/transpose prep, PE-saturating schedule):
  - host pre-frames and pre-transposes the signal into the time-major
    moving-operand layout yt[sb, c, t(128), k(4), row(500)] in bf16, so
    the device does zero data-movement work on the PE: one 512KB DMA per
    (sb, c) lands the rfft moving operand directly
  - rfft as bf16 matmuls with the Hann window folded into the DFT
    matrices; wsin col 0 carries the bin-256 cos column
  - P = Re^2 + Im^2: ACT Square (bf16 out), ph add split DVE/GpSimd
  - irfft uses P as the stationary operand -> acf^T [125 rows, 4 groups,
    256 lags] in one 2-bank PSUM tile; D scaled by 0.25 so adding the
    two channels yields the channel mean of the normalized acf
  - norm: one batched ACT Sqrt + DVE reciprocal over the 4 zero-lag
    columns, then relu(acf*rcc) fused into a single
    scalar_tensor_tensor (mult, max-with-0) per group
  - PE stream is software-pipelined: irfft of superbatch N-1 is emitted
    between the rfft phases of superbatch N so the tensor engine never
    waits on the elementwise chain
"""

import sys

import numpy as np

sys.path.insert(0, "/opt/trn_rl_repo")

B, F, T, C = 4, 50, 20000, 2
NUM_FRAME = 300
LEN_FRAME = 512
LAGS = 256
NBINS = 257
N_CORES = 8
BF_PER_CORE = (B * F) // N_CORES  # 25

FRAMES_PER_SB = 20
TILES_PER_SB = 4
N_SB = NUM_FRAME // FRAMES_PER_SB  # 15
NCOLS = 500  # (20 frames x 25 bf) per channel

STARTS = np.linspace(0, T - LEN_FRAME, NUM_FRAME).astype(np.int64)


def build_weights():
    t = np.arange(LEN_FRAME, dtype=np.float64)
    w = 0.5 - 0.5 * np.cos(2.0 * np.pi * t / LEN_FRAME)  # periodic hann
    ang = 2.0 * np.pi * np.outer(t, np.arange(NBINS)) / LEN_FRAME
    Cm = np.cos(ang) * w[:, None]  # [512, 257]
    Sm = -np.sin(ang) * w[:, None]
    wcos = Cm[:, 0:256].reshape(4, 128, 256).copy()
    wsin = Sm[:, 0:256].reshape(4, 128, 256).copy()
    wsin[:, :, 0] = Cm[:, 256].reshape(4, 128)  # bin-256 cos column
    alpha = 0.25  # folds the channel-mean 0.5 (output scales with sqrt(alpha))
    k = np.arange(NBINS)
    coef = np.full(NBINS, 2.0)
    coef[0] = 1.0
    coef[256] = 1.0
    D = (alpha / LEN_FRAME) * coef[:, None] * np.cos(
        2.0 * np.pi * np.outer(k, np.arange(LAGS)) / LEN_FRAME
    )
    # ph(h0) row 0 carries P[0]+P[256] (sq_i row 0 is P[256] via the wsin
    # col-0 trick and is not masked); compensate exactly in the bin-256 row
    D[256] -= D[0]
    return wcos, wsin, D


def build_nc(n_sb=N_SB):
    from contextlib import ExitStack

    import concourse.bacc as bacc
    import concourse.tile as tile
    from concourse import mybir

    f32 = mybir.dt.float32
    bf16 = mybir.dt.bfloat16
    AF = mybir.ActivationFunctionType
    ALU = mybir.AluOpType

    nc = bacc.Bacc("TRN2", target_bir_lowering=False, debug=False)

    yt_d = nc.dram_tensor(
        "yt", [N_SB, C, 128, TILES_PER_SB, NCOLS], bf16, kind="ExternalInput"
    ).ap()
    wcos_d = nc.dram_tensor("wcos", [4, 128, 256], bf16, kind="ExternalInput").ap()
    wsin_d = nc.dram_tensor("wsin", [4, 128, 256], bf16, kind="ExternalInput").ap()
    dmat_d = nc.dram_tensor("dmat", [NBINS, LAGS], bf16, kind="ExternalInput").ap()
    out = nc.dram_tensor(
        "out", [BF_PER_CORE, NUM_FRAME, LAGS], f32, kind="ExternalOutput"
    ).ap()

    with tile.TileContext(nc) as tc, ExitStack() as ctx:
        consts = ctx.enter_context(tc.tile_pool(name="consts", bufs=1))
        sbp = ctx.enter_context(tc.tile_pool(name="work", bufs=1))
        pp = ctx.enter_context(tc.tile_pool(name="ps", bufs=1, space="PSUM"))

        yt_q = {}  # (s, c) -> yt tile [128, 4, 500]

        def load_yt(s):
            for c in range(C):
                t = sbp.tile([128, TILES_PER_SB, NCOLS], bf16, tag="yt", bufs=8)
                eng = nc.gpsimd if c == 0 else nc.scalar
                eng.dma_start(out=t[:], in_=yt_d[s, c])
                yt_q[(s, c)] = t

        # issue the first moving-operand load before the const DMAs so the
        # first rfft matmul isn't serialized behind them
        load_yt(0)

        # ---- constants (per-k tiles so the first matmul waits on one DMA) ----
        wcos_sb = [
            consts.tile([128, 256], bf16, tag=f"wcos{k}", name=f"wcos_sb{k}")
            for k in range(4)
        ]
        wsin_sb = [
            consts.tile([128, 256], bf16, tag=f"wsin{k}", name=f"wsin_sb{k}")
            for k in range(4)
        ]
        for k in range(4):
            nc.sync.dma_start(out=wcos_sb[k][:], in_=wcos_d[k])
            nc.scalar.dma_start(out=wsin_sb[k][:], in_=wsin_d[k])
        dm0 = consts.tile([128, 256], bf16, tag="dm0")
        dm1 = consts.tile([128, 256], bf16, tag="dm1")
        nc.scalar.dma_start(out=dm0[:], in_=dmat_d[0:128])
        nc.scalar.dma_start(out=dm1[:], in_=dmat_d[128:256])
        zero_b = consts.tile([128, 1], f32, tag="zerob")
        nc.vector.memset(zero_b[:], 0.0)
        eps_b = consts.tile([128, 1], f32, tag="epsb")
        nc.vector.memset(eps_b[:], 1e-30)
        zeros_l = consts.tile([128, LAGS], f32, tag="zerosl")
        nc.vector.memset(zeros_l[:], 0.0)
        # dm2 padded to a full 128-row moving operand (rows 1..127 zero) so
        # the bin-256 matmul can use the full sq_i tile as stationary
        dm2z = consts.tile([128, 256], bf16, tag="dm2z")
        nc.vector.memset(dm2z[:], 0.0)
        nc.sync.dma_start(out=dm2z[0:1, :], in_=dmat_d[256:257])


        ph_q = {}  # (s, c, h) -> ph tile bf16 [128, 500]
        p256_q = {}  # (s, c) -> sq_i(h0) tile (row 0 is P[256])

        def R_phase(s, c, h):
            rp = pp.tile([128, NCOLS], f32, tag="fft", bufs=4)
            ip = pp.tile([128, NCOLS], f32, tag="fft", bufs=4)
            yt = yt_q[(s, c)]
            for k in range(4):
                nc.tensor.matmul(
                    rp[:],
                    wcos_sb[k][:, 128 * h : 128 * h + 128],
                    yt[:, k, :],
                    start=(k == 0),
                    stop=(k == 3),
                )
            for k in range(4):
                nc.tensor.matmul(
                    ip[:],
                    wsin_sb[k][:, 128 * h : 128 * h + 128],
                    yt[:, k, :],
                    start=(k == 0),
                    stop=(k == 3),
                )
            sq_r = sbp.tile([128, NCOLS], bf16, tag="sqr", bufs=6)
            nc.scalar.activation(sq_r[:], rp[:], AF.Square, bias=zero_b[:])
            ph = sbp.tile([128, NCOLS], bf16, tag="ph", bufs=10)
            sq_i = sbp.tile([128, NCOLS], bf16, tag="sqi", bufs=6)
            nc.scalar.activation(sq_i[:], ip[:], AF.Square, bias=zero_b[:])
            if h == 0:
                # sq_i row 0 = Im_h0[0]^2 = P[256] (wsin col 0 carries
                # cos-256). The bin-256 matmul uses the FULL sq_i tile as
                # stationary; dm2z's zero rows 1..127 nullify the other
                # contraction terms. ph row 0 = P[0]+P[256]; dmat row 256 is
                # D[256]-D[0] to compensate exactly.
                nc.vector.tensor_add(ph[:], sq_r[:], sq_i[:])
                p256_q[(s, c)] = sq_i
            else:
                nc.vector.tensor_add(ph[:], sq_r[:], sq_i[:])
            ph_q[(s, c, h)] = ph

        def I_phase(s1, c):
            """irfft matmuls for one channel; norm chain is emitted later."""
            ph0, ph1 = ph_q[(s1, c, 0)], ph_q[(s1, c, 1)]
            p256 = p256_q[(s1, c)]
            acfp = pp.tile([125, 4, LAGS], f32, tag="acf", bufs=2)
            for g in range(4):
                sl = slice(125 * g, 125 * g + 125)
                nc.tensor.matmul(
                    acfp[:, g, :], ph0[:, sl], dm0[:], start=True, stop=False
                )
                nc.tensor.matmul(
                    acfp[:, g, :], ph1[:, sl], dm1[:], start=False, stop=False
                )
                nc.tensor.matmul(
                    acfp[:, g, :], p256[:, sl], dm2z[:], start=False, stop=True
                )
            return acfp

        def norm_phase(c, acfp):
            """sqrt -> recip -> fused relu-scale; emitted after all squares."""
            sqc = sbp.tile([125, 4], f32, tag="sqc", bufs=6)
            nc.scalar.activation(sqc[:], acfp[:, :, 0], AF.Sqrt, bias=eps_b[:125])
            rcc = sbp.tile([125, 4], f32, tag="rcc", bufs=6)
            nc.vector.reciprocal(out=rcc[:], in_=sqc[:])
            nts = []
            for g in range(4):
                nt = sbp.tile([125, LAGS], f32, tag=f"nt{c}", bufs=6)
                nc.vector.scalar_tensor_tensor(
                    out=nt[:],
                    in0=acfp[:, g, :],
                    scalar=rcc[:, g : g + 1],
                    in1=zeros_l[:125, :],
                    op0=ALU.mult,
                    op1=ALU.max,
                )
                nts.append(nt)
            return nts

        # ---- pipeline ----
        load_yt(1)

        def store_sb(s1, nts_c0, nts_c1):
            mt = sbp.tile([125, 4, LAGS], f32, tag="mt", bufs=3)
            m0 = s1 * FRAMES_PER_SB
            for g in range(4):
                nc.gpsimd.tensor_add(mt[:, g, :], nts_c0[g][:], nts_c1[g][:])
                mf = m0 + 5 * g
                nc.sync.dma_start(
                    out=out[:, mf : mf + 5, :].rearrange("bf mm l -> mm bf l"),
                    in_=mt[:, g, :],
                )
            for c in range(C):
                for h in range(2):
                    ph_q.pop((s1, c, h), None)
                p256_q.pop((s1, c), None)
                yt_q.pop((s1, c), None)

        for it in range(n_sb):
            s1 = it - 1
            last = it == n_sb - 1
            if it + 2 < n_sb:
                load_yt(it + 2)

            acf_c0 = acf_c1 = None
            if s1 >= 0:
                acf_c0 = I_phase(s1, 0)
            R_phase(it, 0, 0)
            R_phase(it, 0, 1)
            if s1 >= 0:
                acf_c1 = I_phase(s1, 1)
            if not last:
                R_phase(it, 1, 0)
                if s1 >= 0:
                    nts_c0 = norm_phase(0, acf_c0)
                R_phase(it, 1, 1)
                if s1 >= 0:
                    nts_c1 = norm_phase(1, acf_c1)
                    store_sb(s1, nts_c0, nts_c1)
            else:
                # drain: overlap the final superbatch's irfft/norm with the
                # last R phases so the tail chain is short
                nts_c0 = norm_phase(0, acf_c0)  # frees acf buf for I(it, 0)
                R_phase(it, 1, 0)
                acf_l0 = I_phase(it, 0)
                R_phase(it, 1, 1)
                nts_c1 = norm_phase(1, acf_c1)
                store_sb(s1, nts_c0, nts_c1)
                acf_l1 = I_phase(it, 1)
                nts_l0 = norm_phase(0, acf_l0)
                nts_l1 = norm_phase(1, acf_l1)
                store_sb(it, nts_l0, nts_l1)

    nc.compile()
    return nc


_NC_CACHE = {}


def _get_nc():
    if "nc" not in _NC_CACHE:
        _NC_CACHE["nc"] = build_nc()
    return _NC_CACHE["nc"]


def make_in_maps(nerv):
    import ml_dtypes

    bf16 = ml_dtypes.bfloat16
    xs = nerv.reshape(B * F, T, C)
    idx = STARTS[:, None] + np.arange(LEN_FRAME)  # [300, 512]
    wcos, wsin, dmat = build_weights()
    wcos = wcos.astype(bf16)
    wsin = wsin.astype(bf16)
    dmat = dmat.astype(bf16)
    maps = []
    for i in range(N_CORES):
        xc = xs[BF_PER_CORE * i : BF_PER_CORE * (i + 1)]  # [25, T, 2]
        fr = xc[:, idx, :].astype(bf16)  # [25, 300, 512, 2]
        # -> [sb, c, t, k, m_local, bf]
        yt = fr.reshape(BF_PER_CORE, N_SB, FRAMES_PER_SB, 4, 128, C).transpose(
            1, 5, 4, 3, 2, 0
        )
        yt = np.ascontiguousarray(yt).reshape(N_SB, C, 128, 4, NCOLS)
        maps.append({"yt": yt, "wcos": wcos, "wsin": wsin, "dmat": dmat})
    return maps


def kernel(nervegram, trace=False, **_ignored):
    from concourse.bass_utils import run_bass_kernel_spmd

    nerv = np.ascontiguousarray(np.asarray(nervegram, dtype=np.float32))
    assert nerv.shape == (B, F, T, C)
    in_maps = make_in_maps(nerv)
    nc = _get_nc()
    res = run_bass_kernel_spmd(nc, in_maps, list(range(N_CORES)), trace=trace)
    full = np.concatenate([res.results[i]["out"] for i in range(N_CORES)], axis=0)
    out = full.reshape(B, F, NUM_FRAME, LAGS)
    if trace:
        return out, res
    return out


# revision 40
# speedup vs baseline: 1.0580x; 1.0580x over previous
"""Trainium2 Bass kernel for nn_AutocorrelationCorrelogram.

For nervegram [B=4, F=50, T=20000, C=2]: 300 periodic-Hann-windowed frames
of length 512 per (b,f,c) signal, circular autocorrelation via
Wiener-Khinchin (rfft -> |.|^2 -> irfft), relu, normalize by sqrt(zero
lag), keep 256 lags, mean over channels -> [4, 50, 300, 256].

Sharding: pure data parallel over the 200 (b,f) pairs -> 25 per core x 8
cores (SPMD, no collectives).

v3 design (bf16, host-side frame/transpose prep, PE-saturating schedule):
  - host pre-frames and pre-transposes the signal into the time-major
    moving-operand layout yt[sb, c, t(128), k(4), row(500)] in bf16, so
    the device does zero data-movement work on the PE: one 512KB DMA per
    (sb, c) lands the rfft moving operand directly
  - rfft as bf16 matmuls with the Hann window folded into the DFT
    matrices; wsin col 0 carries the bin-256 cos column
  - P = Re^2 + Im^2: ACT Square (bf16 out), ph add split DVE/GpSimd
  - irfft uses P as the stationary operand -> acf^T [125 rows, 4 groups,
    256 lags] in one 2-bank PSUM tile; D scaled by 0.25 so adding the
    two channels yields the channel mean of the normalized acf
  - norm: one batched ACT Sqrt + DVE reciprocal over the 4 zero-lag
    columns, then relu(acf*rcc) fused into a single
    scalar_tensor_tensor (mult, max-with-0) per group
  - PE stream is software-pipelined: irfft of superbatch N-1 is emitted
    between the rfft phases of superbatch N so the tensor engine never
    waits on the elementwise chain
"""

import sys

import numpy as np

sys.path.insert(0, "/opt/trn_rl_repo")

B, F, T, C = 4, 50, 20000, 2
NUM_FRAME = 300
LEN_FRAME = 512
LAGS = 256
NBINS = 257
N_CORES = 8
BF_PER_CORE = (B * F) // N_CORES  # 25

FRAMES_PER_SB = 20
TILES_PER_SB = 4
N_SB = NUM_FRAME // FRAMES_PER_SB  # 15
NCOLS = 500  # (20 frames x 25 bf) per channel

STARTS = np.linspace(0, T - LEN_FRAME, NUM_FRAME).astype(np.int64)


def build_weights():
    t = np.arange(LEN_FRAME, dtype=np.float64)
    w = 0.5 - 0.5 * np.cos(2.0 * np.pi * t / LEN_FRAME)  # periodic hann
    ang = 2.0 * np.pi * np.outer(t, np.arange(NBINS)) / LEN_FRAME
    Cm = np.cos(ang) * w[:, None]  # [512, 257]
    Sm = -np.sin(ang) * w[:, None]
    wcos = Cm[:, 0:256].reshape(4, 128, 256).copy()
    wsin = Sm[:, 0:256].reshape(4, 128, 256).copy()
    wsin[:, :, 0] = Cm[:, 256].reshape(4, 128)  # bin-256 cos column
    alpha = 0.25  # folds the channel-mean 0.5 (output scales with sqrt(alpha))
    k = np.arange(NBINS)
    coef = np.full(NBINS, 2.0)
    coef[0] = 1.0
    coef[256] = 1.0
    D = (alpha / LEN_FRAME) * coef[:, None] * np.cos(
        2.0 * np.pi * np.outer(k, np.arange(LAGS)) / LEN_FRAME
    )
    # ph(h0) row 0 carries P[0]+P[256] (sq_i row 0 is P[256] via the wsin
    # col-0 trick and is not masked); compensate exactly in the bin-256 row
    D[256] -= D[0]
    return wcos, wsin, D


def build_nc(n_sb=N_SB):
    from contextlib import ExitStack

    import concourse.bacc as bacc
    import concourse.tile as tile
    from concourse import mybir

    f32 = mybir.dt.float32
    bf16 = mybir.dt.bfloat16
    AF = mybir.ActivationFunctionType
    ALU = mybir.AluOpType

    nc = bacc.Bacc("TRN2", target_bir_lowering=False, debug=False)

    yt_d = nc.dram_tensor(
        "yt", [N_SB, C, 128, TILES_PER_SB, NCOLS], bf16, kind="ExternalInput"
    ).ap()
    wcos_d = nc.dram_tensor("wcos", [4, 128, 256], bf16, kind="ExternalInput").ap()
    wsin_d = nc.dram_tensor("wsin", [4, 128, 256], bf16, kind="ExternalInput").ap()
    dmat_d = nc.dram_tensor("dmat", [NBINS, LAGS], bf16, kind="ExternalInput").ap()
    out = nc.dram_tensor(
        "out", [BF_PER_CORE, NUM_FRAME, LAGS], f32, kind="ExternalOutput"
    ).ap()

    with tile.TileContext(nc) as tc, ExitStack() as ctx:
        consts = ctx.enter_context(tc.tile_pool(name="consts", bufs=1))
        sbp = ctx.enter_context(tc.tile_pool(name="work", bufs=1))
        pp = ctx.enter_context(tc.tile_pool(name="ps", bufs=1, space="PSUM"))

        yt_q = {}  # (s, c) -> yt tile [128, 4, 500]

        def load_yt(s):
            # c0 on gpsimd, c1 on sync: never issue DMAs from the scalar
            # queue (it shares the ACT engine with the squares)
            for c in range(C):
                t = sbp.tile([128, TILES_PER_SB, NCOLS], bf16, tag="yt", bufs=8)
                eng = nc.gpsimd if c == 0 else nc.sync
                eng.dma_start(out=t[:], in_=yt_d[s, c])
                yt_q[(s, c)] = t

        # issue the first moving-operand loads before the const DMAs so the
        # first rfft matmuls aren't serialized behind them
        load_yt(0)

        # ---- constants (per-k tiles so the first matmul waits on one DMA) ----
        wcos_sb = [
            consts.tile([128, 256], bf16, tag=f"wcos{k}", name=f"wcos_sb{k}")
            for k in range(4)
        ]
        wsin_sb = [
            consts.tile([128, 256], bf16, tag=f"wsin{k}", name=f"wsin_sb{k}")
            for k in range(4)
        ]
        for k in range(4):
            nc.sync.dma_start(out=wcos_sb[k][:], in_=wcos_d[k])
            nc.scalar.dma_start(out=wsin_sb[k][:], in_=wsin_d[k])
        dm0 = consts.tile([128, 256], bf16, tag="dm0")
        dm1 = consts.tile([128, 256], bf16, tag="dm1")
        nc.scalar.dma_start(out=dm0[:], in_=dmat_d[0:128])
        nc.scalar.dma_start(out=dm1[:], in_=dmat_d[128:256])
        zero_b = consts.tile([128, 1], f32, tag="zerob")
        nc.vector.memset(zero_b[:], 0.0)
        eps_b = consts.tile([128, 1], f32, tag="epsb")
        nc.vector.memset(eps_b[:], 1e-30)
        zeros_l = consts.tile([128, LAGS], f32, tag="zerosl")
        nc.vector.memset(zeros_l[:], 0.0)
        # dm2 padded to a full 128-row moving operand (rows 1..127 zero) so
        # the bin-256 matmul can use the full sq_i tile as stationary
        dm2z = consts.tile([128, 256], bf16, tag="dm2z")
        nc.vector.memset(dm2z[:], 0.0)
        nc.sync.dma_start(out=dm2z[0:1, :], in_=dmat_d[256:257])


        ph_q = {}  # (s, c, h) -> ph tile bf16 [128, 500]
        p256_q = {}  # (s, c) -> sq_i(h0) tile (row 0 is P[256])

        def R_phase(s, c, h):
            rp = pp.tile([128, NCOLS], f32, tag="fft", bufs=4)
            ip = pp.tile([128, NCOLS], f32, tag="fft", bufs=4)
            yt = yt_q[(s, c)]
            for k in range(4):
                nc.tensor.matmul(
                    rp[:],
                    wcos_sb[k][:, 128 * h : 128 * h + 128],
                    yt[:, k, :],
                    start=(k == 0),
                    stop=(k == 3),
                )
            for k in range(4):
                nc.tensor.matmul(
                    ip[:],
                    wsin_sb[k][:, 128 * h : 128 * h + 128],
                    yt[:, k, :],
                    start=(k == 0),
                    stop=(k == 3),
                )
            sq_r = sbp.tile([128, NCOLS], bf16, tag="sqr", bufs=6)
            nc.scalar.activation(sq_r[:], rp[:], AF.Square, bias=zero_b[:])
            ph = sbp.tile([128, NCOLS], bf16, tag="ph", bufs=10)
            sq_i = sbp.tile([128, NCOLS], bf16, tag="sqi", bufs=6)
            nc.scalar.activation(sq_i[:], ip[:], AF.Square, bias=zero_b[:])
            if h == 0:
                # sq_i row 0 = Im_h0[0]^2 = P[256] (wsin col 0 carries
                # cos-256). The bin-256 matmul uses the FULL sq_i tile as
                # stationary; dm2z's zero rows 1..127 nullify the other
                # contraction terms. ph row 0 = P[0]+P[256]; dmat row 256 is
                # D[256]-D[0] to compensate exactly.
                nc.vector.tensor_add(ph[:], sq_r[:], sq_i[:])
                p256_q[(s, c)] = sq_i
            else:
                nc.vector.tensor_add(ph[:], sq_r[:], sq_i[:])
            ph_q[(s, c, h)] = ph

        def I_phase(s1, c):
            """irfft matmuls for one channel; norm chain is emitted later."""
            ph0, ph1 = ph_q[(s1, c, 0)], ph_q[(s1, c, 1)]
            p256 = p256_q[(s1, c)]
            acfp = pp.tile([125, 4, LAGS], f32, tag="acf", bufs=2)
            for g in range(4):
                sl = slice(125 * g, 125 * g + 125)
                nc.tensor.matmul(
                    acfp[:, g, :], ph0[:, sl], dm0[:], start=True, stop=False
                )
                nc.tensor.matmul(
                    acfp[:, g, :], ph1[:, sl], dm1[:], start=False, stop=False
                )
                nc.tensor.matmul(
                    acfp[:, g, :], p256[:, sl], dm2z[:], start=False, stop=True
                )
            return acfp

        def norm_phase(c, acfp, split=False):
            """sqrt -> recip -> fused relu-scale; emitted after all squares.

            split=True runs half the groups as ACT Relu+scale so the drain's
            norm chain parallelizes across ACT and DVE."""
            sqc = sbp.tile([125, 4], f32, tag="sqc", bufs=6)
            nc.scalar.activation(sqc[:], acfp[:, :, 0], AF.Sqrt, bias=eps_b[:125])
            rcc = sbp.tile([125, 4], f32, tag="rcc", bufs=6)
            nc.vector.reciprocal(out=rcc[:], in_=sqc[:])
            nts = []
            for g in range(4):
                nt = sbp.tile([125, LAGS], f32, tag=f"nt{c}", bufs=6)
                if split and g >= 2:
                    nc.scalar.activation(
                        nt[:],
                        acfp[:, g, :],
                        AF.Relu,
                        bias=zero_b[:125],
                        scale=rcc[:, g : g + 1],
                    )
                else:
                    nc.vector.scalar_tensor_tensor(
                        out=nt[:],
                        in0=acfp[:, g, :],
                        scalar=rcc[:, g : g + 1],
                        in1=zeros_l[:125, :],
                        op0=ALU.mult,
                        op1=ALU.max,
                    )
                nts.append(nt)
            return nts

        # ---- pipeline ----
        load_yt(1)

        def store_sb(s1, nts_c0, nts_c1, final=False):
            mt = sbp.tile([125, 4, LAGS], f32, tag="mt", bufs=3)
            m0 = s1 * FRAMES_PER_SB
            for g in range(4):
                nc.gpsimd.tensor_add(mt[:, g, :], nts_c0[g][:], nts_c1[g][:])
                mf = m0 + 5 * g
                eng = nc.sync if (not final or g % 2 == 0) else nc.gpsimd
                eng.dma_start(
                    out=out[:, mf : mf + 5, :].rearrange("bf mm l -> mm bf l"),
                    in_=mt[:, g, :],
                )
            for c in range(C):
                for h in range(2):
                    ph_q.pop((s1, c, h), None)
                p256_q.pop((s1, c), None)
                yt_q.pop((s1, c), None)

        for it in range(n_sb):
            s1 = it - 1
            last = it == n_sb - 1
            if it + 2 < n_sb:
                load_yt(it + 2)

            acf_c0 = acf_c1 = None
            if s1 >= 0:
                acf_c0 = I_phase(s1, 0)
            R_phase(it, 0, 0)
            R_phase(it, 0, 1)
            if s1 >= 0:
                acf_c1 = I_phase(s1, 1)
            if not last:
                R_phase(it, 1, 0)
                if s1 >= 0:
                    nts_c0 = norm_phase(0, acf_c0)
                R_phase(it, 1, 1)
                if s1 >= 0:
                    nts_c1 = norm_phase(1, acf_c1)
                    store_sb(s1, nts_c0, nts_c1)
            else:
                # drain: overlap the final superbatch's irfft/norm with the
                # last R phases so the tail chain is short
                nts_c0 = norm_phase(0, acf_c0)  # frees acf buf for I(it, 0)
                R_phase(it, 1, 0)
                acf_l0 = I_phase(it, 0)
                R_phase(it, 1, 1)
                nts_c1 = norm_phase(1, acf_c1)
                store_sb(s1, nts_c0, nts_c1)
                acf_l1 = I_phase(it, 1)
                nts_l0 = norm_phase(0, acf_l0, split=True)
                nts_l1 = norm_phase(1, acf_l1, split=True)
                store_sb(it, nts_l0, nts_l1, final=True)

    nc.compile()
    return nc


_NC_CACHE = {}


def _get_nc():
    if "nc" not in _NC_CACHE:
        _NC_CACHE["nc"] = build_nc()
    return _NC_CACHE["nc"]


def make_in_maps(nerv):
    import ml_dtypes

    bf16 = ml_dtypes.bfloat16
    xs = nerv.reshape(B * F, T, C)
    idx = STARTS[:, None] + np.arange(LEN_FRAME)  # [300, 512]
    wcos, wsin, dmat = build_weights()
    wcos = wcos.astype(bf16)
    wsin = wsin.astype(bf16)
    dmat = dmat.astype(bf16)
    maps = []
    for i in range(N_CORES):
        xc = xs[BF_PER_CORE * i : BF_PER_CORE * (i + 1)]  # [25, T, 2]
        fr = xc[:, idx, :].astype(bf16)  # [25, 300, 512, 2]
        # -> [sb, c, t, k, m_local, bf]
        yt = fr.reshape(BF_PER_CORE, N_SB, FRAMES_PER_SB, 4, 128, C).transpose(
            1, 5, 4, 3, 2, 0
        )
        yt = np.ascontiguousarray(yt).reshape(N_SB, C, 128, 4, NCOLS)
        maps.append({"yt": yt, "wcos": wcos, "wsin": wsin, "dmat": dmat})
    return maps


def kernel(nervegram, trace=False, **_ignored):
    from concourse.bass_utils import run_bass_kernel_spmd

    nerv = np.ascontiguousarray(np.asarray(nervegram, dtype=np.float32))
    assert nerv.shape == (B, F, T, C)
    in_maps = make_in_maps(nerv)
    nc = _get_nc()
    res = run_bass_kernel_spmd(nc, in_maps, list(range(N_CORES)), trace=trace)
    full = np.concatenate([res.results[i]["out"] for i in range(N_CORES)], axis=0)
    out = full.reshape(B, F, NUM_FRAME, LAGS)
    if trace:
        return out, res
    return out


# revision 41
# speedup vs baseline: 1.0696x; 1.0109x over previous
"""Trainium2 Bass kernel for nn_AutocorrelationCorrelogram.

For nervegram [B=4, F=50, T=20000, C=2]: 300 periodic-Hann-windowed frames
of length 512 per (b,f,c) signal, circular autocorrelation via
Wiener-Khinchin (rfft -> |.|^2 -> irfft), relu, normalize by sqrt(zero
lag), keep 256 lags, mean over channels -> [4, 50, 300, 256].

Sharding: pure data parallel over the 200 (b,f) pairs -> 25 per core x 8
cores (SPMD, no collectives).

v3 design (bf16, host-side frame/transpose prep, PE-saturating schedule):
  - host pre-frames and pre-transposes the signal into the time-major
    moving-operand layout yt[sb, c, t(128), k(4), row(500)] in bf16, so
    the device does zero data-movement work on the PE: one 512KB DMA per
    (sb, c) lands the rfft moving operand directly
  - rfft as bf16 matmuls with the Hann window folded into the DFT
    matrices; wsin col 0 carries the bin-256 cos column
  - P = Re^2 + Im^2: ACT Square (bf16 out), ph add split DVE/GpSimd
  - irfft uses P as the stationary operand -> acf^T [125 rows, 4 groups,
    256 lags] in one 2-bank PSUM tile; D scaled by 0.25 so adding the
    two channels yields the channel mean of the normalized acf
  - norm: one batched ACT Sqrt + DVE reciprocal over the 4 zero-lag
    columns, then relu(acf*rcc) fused into a single
    scalar_tensor_tensor (mult, max-with-0) per group
  - PE stream is software-pipelined: irfft of superbatch N-1 is emitted
    between the rfft phases of superbatch N so the tensor engine never
    waits on the elementwise chain
"""

import sys

import numpy as np

sys.path.insert(0, "/opt/trn_rl_repo")

B, F, T, C = 4, 50, 20000, 2
NUM_FRAME = 300
LEN_FRAME = 512
LAGS = 256
NBINS = 257
N_CORES = 8
BF_PER_CORE = (B * F) // N_CORES  # 25

FRAMES_PER_SB = 20
TILES_PER_SB = 4
N_SB = NUM_FRAME // FRAMES_PER_SB  # 15
NCOLS = 500  # (20 frames x 25 bf) per channel

STARTS = np.linspace(0, T - LEN_FRAME, NUM_FRAME).astype(np.int64)


def build_weights():
    t = np.arange(LEN_FRAME, dtype=np.float64)
    w = 0.5 - 0.5 * np.cos(2.0 * np.pi * t / LEN_FRAME)  # periodic hann
    ang = 2.0 * np.pi * np.outer(t, np.arange(NBINS)) / LEN_FRAME
    Cm = np.cos(ang) * w[:, None]  # [512, 257]
    Sm = -np.sin(ang) * w[:, None]
    wcos = Cm[:, 0:256].reshape(4, 128, 256).copy()
    wsin = Sm[:, 0:256].reshape(4, 128, 256).copy()
    wsin[:, :, 0] = Cm[:, 256].reshape(4, 128)  # bin-256 cos column
    alpha = 0.25  # folds the channel-mean 0.5 (output scales with sqrt(alpha))
    k = np.arange(NBINS)
    coef = np.full(NBINS, 2.0)
    coef[0] = 1.0
    coef[256] = 1.0
    D = (alpha / LEN_FRAME) * coef[:, None] * np.cos(
        2.0 * np.pi * np.outer(k, np.arange(LAGS)) / LEN_FRAME
    )
    # ph(h0) row 0 carries P[0]+P[256] (sq_i row 0 is P[256] via the wsin
    # col-0 trick and is not masked); compensate exactly in the bin-256 row
    D[256] -= D[0]
    return wcos, wsin, D


def build_nc(n_sb=N_SB):
    from contextlib import ExitStack

    import concourse.bacc as bacc
    import concourse.tile as tile
    from concourse import mybir

    f32 = mybir.dt.float32
    bf16 = mybir.dt.bfloat16
    AF = mybir.ActivationFunctionType
    ALU = mybir.AluOpType

    nc = bacc.Bacc("TRN2", target_bir_lowering=False, debug=False)

    yt_d = nc.dram_tensor(
        "yt", [N_SB, C, 128, TILES_PER_SB, NCOLS], bf16, kind="ExternalInput"
    ).ap()
    wcos_d = nc.dram_tensor("wcos", [4, 128, 256], bf16, kind="ExternalInput").ap()
    wsin_d = nc.dram_tensor("wsin", [4, 128, 256], bf16, kind="ExternalInput").ap()
    dmat_d = nc.dram_tensor("dmat", [NBINS, LAGS], bf16, kind="ExternalInput").ap()
    out = nc.dram_tensor(
        "out", [BF_PER_CORE, NUM_FRAME, LAGS], f32, kind="ExternalOutput"
    ).ap()

    with tile.TileContext(nc) as tc, ExitStack() as ctx:
        consts = ctx.enter_context(tc.tile_pool(name="consts", bufs=1))
        sbp = ctx.enter_context(tc.tile_pool(name="work", bufs=1))
        pp = ctx.enter_context(tc.tile_pool(name="ps", bufs=1, space="PSUM"))

        yt_q = {}  # (s, c) -> yt tile [128, 4, 500]

        def load_yt(s):
            # c0 on gpsimd, c1 on sync: never issue DMAs from the scalar
            # queue (it shares the ACT engine with the squares)
            for c in range(C):
                t = sbp.tile([128, TILES_PER_SB, NCOLS], bf16, tag="yt", bufs=8)
                eng = nc.gpsimd if c == 0 else nc.sync
                eng.dma_start(out=t[:], in_=yt_d[s, c])
                yt_q[(s, c)] = t

        # issue the first moving-operand loads before the const DMAs so the
        # first rfft matmuls aren't serialized behind them
        load_yt(0)

        # ---- constants (per-k tiles, interleaved across two queues in
        # first-use order so the first R phase's k-loop never outruns them) ----
        wcos_sb = [
            consts.tile([128, 256], bf16, tag=f"wcos{k}", name=f"wcos_sb{k}")
            for k in range(4)
        ]
        wsin_sb = [
            consts.tile([128, 256], bf16, tag=f"wsin{k}", name=f"wsin_sb{k}")
            for k in range(4)
        ]
        for k in range(4):
            eng = nc.sync if k % 2 == 0 else nc.scalar
            eng.dma_start(out=wcos_sb[k][:], in_=wcos_d[k])
        for k in range(4):
            eng = nc.sync if k % 2 == 0 else nc.scalar
            eng.dma_start(out=wsin_sb[k][:], in_=wsin_d[k])
        dm0 = consts.tile([128, 256], bf16, tag="dm0")
        dm1 = consts.tile([128, 256], bf16, tag="dm1")
        nc.scalar.dma_start(out=dm0[:], in_=dmat_d[0:128])
        nc.scalar.dma_start(out=dm1[:], in_=dmat_d[128:256])
        zero_b = consts.tile([128, 1], f32, tag="zerob")
        nc.vector.memset(zero_b[:], 0.0)
        eps_b = consts.tile([128, 1], f32, tag="epsb")
        nc.vector.memset(eps_b[:], 1e-30)
        zeros_l = consts.tile([128, LAGS], f32, tag="zerosl")
        nc.vector.memset(zeros_l[:], 0.0)
        # dm2 padded to a full 128-row moving operand (rows 1..127 zero) so
        # the bin-256 matmul can use the full sq_i tile as stationary
        dm2z = consts.tile([128, 256], bf16, tag="dm2z")
        nc.vector.memset(dm2z[:], 0.0)
        nc.sync.dma_start(out=dm2z[0:1, :], in_=dmat_d[256:257])


        ph_q = {}  # (s, c, h) -> ph tile bf16 [128, 500]
        p256_q = {}  # (s, c) -> sq_i(h0) tile (row 0 is P[256])

        def R_phase(s, c, h):
            rp = pp.tile([128, NCOLS], f32, tag="fft", bufs=4)
            ip = pp.tile([128, NCOLS], f32, tag="fft", bufs=4)
            yt = yt_q[(s, c)]
            for k in range(4):
                nc.tensor.matmul(
                    rp[:],
                    wcos_sb[k][:, 128 * h : 128 * h + 128],
                    yt[:, k, :],
                    start=(k == 0),
                    stop=(k == 3),
                )
            for k in range(4):
                nc.tensor.matmul(
                    ip[:],
                    wsin_sb[k][:, 128 * h : 128 * h + 128],
                    yt[:, k, :],
                    start=(k == 0),
                    stop=(k == 3),
                )
            sq_r = sbp.tile([128, NCOLS], bf16, tag="sqr", bufs=6)
            nc.scalar.activation(sq_r[:], rp[:], AF.Square, bias=zero_b[:])
            ph = sbp.tile([128, NCOLS], bf16, tag="ph", bufs=10)
            sq_i = sbp.tile([128, NCOLS], bf16, tag="sqi", bufs=6)
            nc.scalar.activation(sq_i[:], ip[:], AF.Square, bias=zero_b[:])
            if h == 0:
                # sq_i row 0 = Im_h0[0]^2 = P[256] (wsin col 0 carries
                # cos-256). The bin-256 matmul uses the FULL sq_i tile as
                # stationary; dm2z's zero rows 1..127 nullify the other
                # contraction terms. ph row 0 = P[0]+P[256]; dmat row 256 is
                # D[256]-D[0] to compensate exactly.
                nc.vector.tensor_add(ph[:], sq_r[:], sq_i[:])
                p256_q[(s, c)] = sq_i
            else:
                nc.vector.tensor_add(ph[:], sq_r[:], sq_i[:])
            ph_q[(s, c, h)] = ph

        def I_phase(s1, c):
            """irfft matmuls for one channel; norm chain is emitted later."""
            ph0, ph1 = ph_q[(s1, c, 0)], ph_q[(s1, c, 1)]
            p256 = p256_q[(s1, c)]
            acfp = pp.tile([125, 4, LAGS], f32, tag="acf", bufs=2)
            for g in range(4):
                sl = slice(125 * g, 125 * g + 125)
                nc.tensor.matmul(
                    acfp[:, g, :], ph0[:, sl], dm0[:], start=True, stop=False
                )
                nc.tensor.matmul(
                    acfp[:, g, :], ph1[:, sl], dm1[:], start=False, stop=False
                )
                nc.tensor.matmul(
                    acfp[:, g, :], p256[:, sl], dm2z[:], start=False, stop=True
                )
            return acfp

        def norm_phase(c, acfp, split=False):
            """sqrt -> recip -> fused relu-scale; emitted after all squares.

            split=True runs half the groups as ACT Relu+scale so the drain's
            norm chain parallelizes across ACT and DVE."""
            sqc = sbp.tile([125, 4], f32, tag="sqc", bufs=6)
            nc.scalar.activation(sqc[:], acfp[:, :, 0], AF.Sqrt, bias=eps_b[:125])
            rcc = sbp.tile([125, 4], f32, tag="rcc", bufs=6)
            nc.vector.reciprocal(out=rcc[:], in_=sqc[:])
            nts = []
            for g in range(4):
                nt = sbp.tile([125, LAGS], f32, tag=f"nt{c}", bufs=6)
                if split and g >= 2:
                    nc.scalar.activation(
                        nt[:],
                        acfp[:, g, :],
                        AF.Relu,
                        bias=zero_b[:125],
                        scale=rcc[:, g : g + 1],
                    )
                else:
                    nc.vector.scalar_tensor_tensor(
                        out=nt[:],
                        in0=acfp[:, g, :],
                        scalar=rcc[:, g : g + 1],
                        in1=zeros_l[:125, :],
                        op0=ALU.mult,
                        op1=ALU.max,
                    )
                nts.append(nt)
            return nts

        # ---- pipeline ----
        load_yt(1)

        def store_sb(s1, nts_c0, nts_c1, final=False):
            mt = sbp.tile([125, 4, LAGS], f32, tag="mt", bufs=3)
            m0 = s1 * FRAMES_PER_SB
            for g in range(4):
                nc.gpsimd.tensor_add(mt[:, g, :], nts_c0[g][:], nts_c1[g][:])
                mf = m0 + 5 * g
                eng = nc.sync if (not final or g % 2 == 0) else nc.gpsimd
                eng.dma_start(
                    out=out[:, mf : mf + 5, :].rearrange("bf mm l -> mm bf l"),
                    in_=mt[:, g, :],
                )
            for c in range(C):
                for h in range(2):
                    ph_q.pop((s1, c, h), None)
                p256_q.pop((s1, c), None)
                yt_q.pop((s1, c), None)

        for it in range(n_sb):
            s1 = it - 1
            last = it == n_sb - 1
            if it + 2 < n_sb:
                load_yt(it + 2)

            acf_c0 = acf_c1 = None
            if s1 >= 0:
                acf_c0 = I_phase(s1, 0)
            R_phase(it, 0, 0)
            R_phase(it, 0, 1)
            if s1 >= 0:
                acf_c1 = I_phase(s1, 1)
            if not last:
                R_phase(it, 1, 0)
                if s1 >= 0:
                    nts_c0 = norm_phase(0, acf_c0)
                R_phase(it, 1, 1)
                if s1 >= 0:
                    nts_c1 = norm_phase(1, acf_c1)
                    store_sb(s1, nts_c0, nts_c1)
            else:
                # drain: overlap the final superbatch's irfft/norm with the
                # last R phases so the tail chain is short
                nts_c0 = norm_phase(0, acf_c0)  # frees acf buf for I(it, 0)
                R_phase(it, 1, 0)
                acf_l0 = I_phase(it, 0)
                R_phase(it, 1, 1)
                nts_c1 = norm_phase(1, acf_c1)
                store_sb(s1, nts_c0, nts_c1)
                acf_l1 = I_phase(it, 1)
                nts_l0 = norm_phase(0, acf_l0, split=True)
                nts_l1 = norm_phase(1, acf_l1, split=True)
                store_sb(it, nts_l0, nts_l1, final=True)

    nc.compile()
    return nc


_NC_CACHE = {}


def _get_nc():
    if "nc" not in _NC_CACHE:
        _NC_CACHE["nc"] = build_nc()
    return _NC_CACHE["nc"]


def make_in_maps(nerv):
    import ml_dtypes

    bf16 = ml_dtypes.bfloat16
    xs = nerv.reshape(B * F, T, C)
    idx = STARTS[:, None] + np.arange(LEN_FRAME)  # [300, 512]
    wcos, wsin, dmat = build_weights()
    wcos = wcos.astype(bf16)
    wsin = wsin.astype(bf16)
    dmat = dmat.astype(bf16)
    maps = []
    for i in range(N_CORES):
        xc = xs[BF_PER_CORE * i : BF_PER_CORE * (i + 1)]  # [25, T, 2]
        fr = xc[:, idx, :].astype(bf16)  # [25, 300, 512, 2]
        # -> [sb, c, t, k, m_local, bf]
        yt = fr.reshape(BF_PER_CORE, N_SB, FRAMES_PER_SB, 4, 128, C).transpose(
            1, 5, 4, 3, 2, 0
        )
        yt = np.ascontiguousarray(yt).reshape(N_SB, C, 128, 4, NCOLS)
        maps.append({"yt": yt, "wcos": wcos, "wsin": wsin, "dmat": dmat})
    return maps


def kernel(nervegram, trace=False, **_ignored):
    from concourse.bass_utils import run_bass_kernel_spmd

    nerv = np.ascontiguousarray(np.asarray(nervegram, dtype=np.float32))
    assert nerv.shape == (B, F, T, C)
    in_maps = make_in_maps(nerv)
    nc = _get_nc()
    res = run_bass_kernel_spmd(nc, in_maps, list(range(N_CORES)), trace=trace)
    full = np.concatenate([res.results[i]["out"] for i in range(N_CORES)], axis=0)
    out = full.reshape(B, F, NUM_FRAME, LAGS)
    if trace:
        return out, res
    return out


# revision 44
# speedup vs baseline: 1.0860x; 1.0153x over previous
"""Trainium2 Bass kernel for nn_AutocorrelationCorrelogram.

For nervegram [B=4, F=50, T=20000, C=2]: 300 periodic-Hann-windowed frames
of length 512 per (b,f,c) signal, circular autocorrelation via
Wiener-Khinchin (rfft -> |.|^2 -> irfft), relu, normalize by sqrt(zero
lag), keep 256 lags, mean over channels -> [4, 50, 300, 256].

Sharding: pure data parallel over the 200 (b,f) pairs -> 25 per core x 8
cores (SPMD, no collectives).

v3 design (bf16, host-side frame/transpose prep, PE-saturating schedule):
  - host pre-frames and pre-transposes the signal into the time-major
    moving-operand layout yt[sb, c, t(128), k(4), row(500)] in bf16, so
    the device does zero data-movement work on the PE: one 512KB DMA per
    (sb, c) lands the rfft moving operand directly
  - rfft as bf16 matmuls with the Hann window folded into the DFT
    matrices; wsin col 0 carries the bin-256 cos column
  - P = Re^2 + Im^2: ACT Square (bf16 out), ph add split DVE/GpSimd
  - irfft uses P as the stationary operand -> acf^T [125 rows, 4 groups,
    256 lags] in one 2-bank PSUM tile; D scaled by 0.25 so adding the
    two channels yields the channel mean of the normalized acf
  - norm: one batched ACT Sqrt + DVE reciprocal over the 4 zero-lag
    columns, then relu(acf*rcc) fused into a single
    scalar_tensor_tensor (mult, max-with-0) per group
  - PE stream is software-pipelined: irfft of superbatch N-1 is emitted
    between the rfft phases of superbatch N so the tensor engine never
    waits on the elementwise chain
"""

import sys

import numpy as np

sys.path.insert(0, "/opt/trn_rl_repo")

B, F, T, C = 4, 50, 20000, 2
NUM_FRAME = 300
LEN_FRAME = 512
LAGS = 256
NBINS = 257
N_CORES = 8
BF_PER_CORE = (B * F) // N_CORES  # 25

FRAMES_PER_SB = 20
TILES_PER_SB = 4
N_SB = NUM_FRAME // FRAMES_PER_SB  # 15
NCOLS = 500  # (20 frames x 25 bf) per channel

STARTS = np.linspace(0, T - LEN_FRAME, NUM_FRAME).astype(np.int64)


def build_weights():
    t = np.arange(LEN_FRAME, dtype=np.float64)
    w = 0.5 - 0.5 * np.cos(2.0 * np.pi * t / LEN_FRAME)  # periodic hann
    ang = 2.0 * np.pi * np.outer(t, np.arange(NBINS)) / LEN_FRAME
    Cm = np.cos(ang) * w[:, None]  # [512, 257]
    Sm = -np.sin(ang) * w[:, None]
    wcos = Cm[:, 0:256].reshape(4, 128, 256).copy()
    wsin = Sm[:, 0:256].reshape(4, 128, 256).copy()
    wsin[:, :, 0] = Cm[:, 256].reshape(4, 128)  # bin-256 cos column
    alpha = 0.25  # folds the channel-mean 0.5 (output scales with sqrt(alpha))
    k = np.arange(NBINS)
    coef = np.full(NBINS, 2.0)
    coef[0] = 1.0
    coef[256] = 1.0
    D = (alpha / LEN_FRAME) * coef[:, None] * np.cos(
        2.0 * np.pi * np.outer(k, np.arange(LAGS)) / LEN_FRAME
    )
    # ph(h0) row 0 carries P[0]+P[256] (sq_i row 0 is P[256] via the wsin
    # col-0 trick and is not masked); compensate exactly in the bin-256 row
    D[256] -= D[0]
    return wcos, wsin, D


def build_nc(n_sb=N_SB):
    from contextlib import ExitStack

    import concourse.bacc as bacc
    import concourse.tile as tile
    from concourse import mybir

    f32 = mybir.dt.float32
    bf16 = mybir.dt.bfloat16
    AF = mybir.ActivationFunctionType
    ALU = mybir.AluOpType

    nc = bacc.Bacc("TRN2", target_bir_lowering=False, debug=False)

    yt_d = nc.dram_tensor(
        "yt", [N_SB, C, 128, TILES_PER_SB, NCOLS], bf16, kind="ExternalInput"
    ).ap()
    wcos_d = nc.dram_tensor("wcos", [4, 128, 256], bf16, kind="ExternalInput").ap()
    wsin_d = nc.dram_tensor("wsin", [4, 128, 256], bf16, kind="ExternalInput").ap()
    dmat_d = nc.dram_tensor("dmat", [NBINS, LAGS], bf16, kind="ExternalInput").ap()
    out = nc.dram_tensor(
        "out", [BF_PER_CORE, NUM_FRAME, LAGS], f32, kind="ExternalOutput"
    ).ap()

    with tile.TileContext(nc) as tc, ExitStack() as ctx:
        consts = ctx.enter_context(tc.tile_pool(name="consts", bufs=1))
        sbp = ctx.enter_context(tc.tile_pool(name="work", bufs=1))
        pp = ctx.enter_context(tc.tile_pool(name="ps", bufs=1, space="PSUM"))

        yt_q = {}  # (s, c) -> yt tile [128, 4, 500]

        def load_yt(s, split=False):
            # c0 on gpsimd, c1 on sync: never issue DMAs from the scalar
            # queue (it shares the ACT engine with the squares)
            for c in range(C):
                eng = nc.gpsimd if c == 0 else nc.sync
                if split:
                    # per-k tiles: the first matmul waits on one 128KB DMA
                    # instead of the full 512KB load
                    ts = []
                    for k in range(TILES_PER_SB):
                        tk = sbp.tile(
                            [128, NCOLS], bf16, tag="yt0", bufs=16,
                            name=f"yt0_{c}_{k}",
                        )
                        eng.dma_start(out=tk[:], in_=yt_d[s, c, :, k, :])
                        ts.append(tk)
                    yt_q[(s, c)] = ts
                else:
                    t = sbp.tile([128, TILES_PER_SB, NCOLS], bf16, tag="yt", bufs=8)
                    eng.dma_start(out=t[:], in_=yt_d[s, c])
                    yt_q[(s, c)] = t

        # issue the first moving-operand loads before the const DMAs so the
        # first rfft matmuls aren't serialized behind them
        load_yt(0, split=True)

        # ---- constants (per-k tiles, interleaved across two queues in
        # first-use order so the first R phase's k-loop never outruns them) ----
        wcos_sb = [
            consts.tile([128, 256], bf16, tag=f"wcos{k}", name=f"wcos_sb{k}")
            for k in range(4)
        ]
        wsin_sb = [
            consts.tile([128, 256], bf16, tag=f"wsin{k}", name=f"wsin_sb{k}")
            for k in range(4)
        ]
        for k in range(4):
            eng = nc.sync if k % 2 == 0 else nc.scalar
            eng.dma_start(out=wcos_sb[k][:], in_=wcos_d[k])
        for k in range(4):
            eng = nc.sync if k % 2 == 0 else nc.scalar
            eng.dma_start(out=wsin_sb[k][:], in_=wsin_d[k])
        dm0 = consts.tile([128, 256], bf16, tag="dm0")
        dm1 = consts.tile([128, 256], bf16, tag="dm1")
        nc.scalar.dma_start(out=dm0[:], in_=dmat_d[0:128])
        nc.scalar.dma_start(out=dm1[:], in_=dmat_d[128:256])
        zero_b = consts.tile([128, 1], f32, tag="zerob")
        nc.vector.memset(zero_b[:], 0.0)
        eps_b = consts.tile([128, 1], f32, tag="epsb")
        nc.vector.memset(eps_b[:], 1e-30)
        zeros_l = consts.tile([128, LAGS], f32, tag="zerosl")
        nc.vector.memset(zeros_l[:], 0.0)
        # dm2 padded to a full 128-row moving operand (rows 1..127 zero) so
        # the bin-256 matmul can use the full sq_i tile as stationary
        dm2z = consts.tile([128, 256], bf16, tag="dm2z")
        nc.vector.memset(dm2z[:], 0.0)
        nc.sync.dma_start(out=dm2z[0:1, :], in_=dmat_d[256:257])


        ph_q = {}  # (s, c, h) -> ph tile bf16 [128, 500]
        p256_q = {}  # (s, c) -> sq_i(h0) tile (row 0 is P[256])

        def R_phase(s, c, h):
            rp = pp.tile([128, NCOLS], f32, tag="fft", bufs=4)
            ip = pp.tile([128, NCOLS], f32, tag="fft", bufs=4)
            yt = yt_q[(s, c)]
            def mov(k):
                return yt[k][:] if isinstance(yt, list) else yt[:, k, :]

            for k in range(4):
                nc.tensor.matmul(
                    rp[:],
                    wcos_sb[k][:, 128 * h : 128 * h + 128],
                    mov(k),
                    start=(k == 0),
                    stop=(k == 3),
                )
            for k in range(4):
                nc.tensor.matmul(
                    ip[:],
                    wsin_sb[k][:, 128 * h : 128 * h + 128],
                    mov(k),
                    start=(k == 0),
                    stop=(k == 3),
                )
            sq_r = sbp.tile([128, NCOLS], bf16, tag="sqr", bufs=6)
            nc.scalar.activation(sq_r[:], rp[:], AF.Square, bias=zero_b[:])
            ph = sbp.tile([128, NCOLS], bf16, tag="ph", bufs=10)
            sq_i = sbp.tile([128, NCOLS], bf16, tag="sqi", bufs=6)
            nc.scalar.activation(sq_i[:], ip[:], AF.Square, bias=zero_b[:])
            if h == 0:
                # sq_i row 0 = Im_h0[0]^2 = P[256] (wsin col 0 carries
                # cos-256). The bin-256 matmul uses the FULL sq_i tile as
                # stationary; dm2z's zero rows 1..127 nullify the other
                # contraction terms. ph row 0 = P[0]+P[256]; dmat row 256 is
                # D[256]-D[0] to compensate exactly.
                nc.vector.tensor_add(ph[:], sq_r[:], sq_i[:])
                p256_q[(s, c)] = sq_i
            else:
                nc.vector.tensor_add(ph[:], sq_r[:], sq_i[:])
            ph_q[(s, c, h)] = ph

        def I_phase(s1, c):
            """irfft matmuls for one channel; norm chain is emitted later."""
            ph0, ph1 = ph_q[(s1, c, 0)], ph_q[(s1, c, 1)]
            p256 = p256_q[(s1, c)]
            acfp = pp.tile([125, 4, LAGS], f32, tag="acf", bufs=2)
            for g in range(4):
                sl = slice(125 * g, 125 * g + 125)
                nc.tensor.matmul(
                    acfp[:, g, :], ph0[:, sl], dm0[:], start=True, stop=False
                )
                nc.tensor.matmul(
                    acfp[:, g, :], ph1[:, sl], dm1[:], start=False, stop=False
                )
                nc.tensor.matmul(
                    acfp[:, g, :], p256[:, sl], dm2z[:], start=False, stop=True
                )
            return acfp

        def norm_phase(c, acfp, split=False):
            """sqrt -> recip -> fused relu-scale; emitted after all squares.

            split=True runs half the groups as ACT Relu+scale so the drain's
            norm chain parallelizes across ACT and DVE."""
            sqc = sbp.tile([125, 4], f32, tag="sqc", bufs=6)
            nc.scalar.activation(sqc[:], acfp[:, :, 0], AF.Sqrt, bias=eps_b[:125])
            rcc = sbp.tile([125, 4], f32, tag="rcc", bufs=6)
            nc.vector.reciprocal(out=rcc[:], in_=sqc[:])
            nts = []
            for g in range(4):
                nt = sbp.tile([125, LAGS], f32, tag=f"nt{c}", bufs=6)
                if split and g >= 2:
                    nc.scalar.activation(
                        nt[:],
                        acfp[:, g, :],
                        AF.Relu,
                        bias=zero_b[:125],
                        scale=rcc[:, g : g + 1],
                    )
                else:
                    nc.vector.scalar_tensor_tensor(
                        out=nt[:],
                        in0=acfp[:, g, :],
                        scalar=rcc[:, g : g + 1],
                        in1=zeros_l[:125, :],
                        op0=ALU.mult,
                        op1=ALU.max,
                    )
                nts.append(nt)
            return nts

        # ---- pipeline ----
        load_yt(1)

        def store_sb(s1, nts_c0, nts_c1, final=False):
            mt = sbp.tile([125, 4, LAGS], f32, tag="mt", bufs=3)
            m0 = s1 * FRAMES_PER_SB
            for g in range(4):
                eng_add = nc.gpsimd if g % 2 == 0 else nc.vector
                eng_add.tensor_add(mt[:, g, :], nts_c0[g][:], nts_c1[g][:])
                mf = m0 + 5 * g
                eng = nc.sync if (not final or g % 2 == 0) else nc.gpsimd
                eng.dma_start(
                    out=out[:, mf : mf + 5, :].rearrange("bf mm l -> mm bf l"),
                    in_=mt[:, g, :],
                )
            for c in range(C):
                for h in range(2):
                    ph_q.pop((s1, c, h), None)
                p256_q.pop((s1, c), None)
                yt_q.pop((s1, c), None)

        for it in range(n_sb):
            s1 = it - 1
            last = it == n_sb - 1
            if it + 2 < n_sb:
                load_yt(it + 2)

            acf_c0 = acf_c1 = None
            if s1 >= 0:
                acf_c0 = I_phase(s1, 0)
            R_phase(it, 0, 0)
            R_phase(it, 0, 1)
            if s1 >= 0:
                acf_c1 = I_phase(s1, 1)
            if not last:
                R_phase(it, 1, 0)
                if s1 >= 0:
                    nts_c0 = norm_phase(0, acf_c0)
                R_phase(it, 1, 1)
                if s1 >= 0:
                    nts_c1 = norm_phase(1, acf_c1)
                    store_sb(s1, nts_c0, nts_c1)
            else:
                # drain: overlap the final superbatch's irfft/norm with the
                # last R phases so the tail chain is short
                nts_c0 = norm_phase(0, acf_c0)  # frees acf buf for I(it, 0)
                R_phase(it, 1, 0)
                acf_l0 = I_phase(it, 0)
                R_phase(it, 1, 1)
                nts_c1 = norm_phase(1, acf_c1)
                store_sb(s1, nts_c0, nts_c1)
                acf_l1 = I_phase(it, 1)
                nts_l0 = norm_phase(0, acf_l0, split=True)
                nts_l1 = norm_phase(1, acf_l1, split=True)
                store_sb(it, nts_l0, nts_l1, final=True)

    nc.compile()
    return nc


_NC_CACHE = {}


def _get_nc():
    if "nc" not in _NC_CACHE:
        _NC_CACHE["nc"] = build_nc()
    return _NC_CACHE["nc"]


def make_in_maps(nerv):
    import ml_dtypes

    bf16 = ml_dtypes.bfloat16
    xs = nerv.reshape(B * F, T, C)
    idx = STARTS[:, None] + np.arange(LEN_FRAME)  # [300, 512]
    wcos, wsin, dmat = build_weights()
    wcos = wcos.astype(bf16)
    wsin = wsin.astype(bf16)
    dmat = dmat.astype(bf16)
    maps = []
    for i in range(N_CORES):
        xc = xs[BF_PER_CORE * i : BF_PER_CORE * (i + 1)]  # [25, T, 2]
        fr = xc[:, idx, :].astype(bf16)  # [25, 300, 512, 2]
        # -> [sb, c, t, k, m_local, bf]
        yt = fr.reshape(BF_PER_CORE, N_SB, FRAMES_PER_SB, 4, 128, C).transpose(
            1, 5, 4, 3, 2, 0
        )
        yt = np.ascontiguousarray(yt).reshape(N_SB, C, 128, 4, NCOLS)
        maps.append({"yt": yt, "wcos": wcos, "wsin": wsin, "dmat": dmat})
    return maps


def kernel(nervegram, trace=False, **_ignored):
    from concourse.bass_utils import run_bass_kernel_spmd

    nerv = np.ascontiguousarray(np.asarray(nervegram, dtype=np.float32))
    assert nerv.shape == (B, F, T, C)
    in_maps = make_in_maps(nerv)
    nc = _get_nc()
    res = run_bass_kernel_spmd(nc, in_maps, list(range(N_CORES)), trace=trace)
    full = np.concatenate([res.results[i]["out"] for i in range(N_CORES)], axis=0)
    out = full.reshape(B, F, NUM_FRAME, LAGS)
    if trace:
        return out, res
    return out
